# revision 1
# baseline (speedup 1.0000x reference)
"""Trainium2 Bass kernel for nn_BMMS8TS8NS8T: batched int8-valued GEMM with
dequant/requant, sharded head-parallel across 8 NeuronCores.

Reference semantics (jax CPU, fp32):
    a = x.float() - a_zp          # [B,H,S,D]  int8-valued
    b = y.float() - b_zp          # [B,H,D,T]
    q = a @ b                     # exact integers (|q| <= 64*131*132 < 2^24)
    v = fl(fl(q * s) + out_zp),   s = fl(alpha / o_alpha)
    out = trunc(clip(v, -128, 127)).astype(int8)   # trunc toward zero

Device strategy per core (12 heads = (B*H)/8, head parallel, no cross-core
communication):
  - host pre-dequantizes inputs to bf16 (exact: all values are integers with
    |v| <= 132 < 256, exactly representable in bf16) and pre-transposes /
    permutes x so the stationary matmul operand needs no on-device transpose
  - TensorE: K=64 matmuls, two heads packed in the 128-row PE array via row
    tiling (tile_position (0,0)/(64,0)); fp32 PSUM accumulation is exact
  - requantization is a two-pass scheme (exact trunc toward zero cannot be a
    single affine op + RNE convert: the trunc bin at 0 is double-width):
      pass1:  A_i16 = floor(v)   (clipped by i16 saturation far out)
      pass2:  out_i8 = sat_i8(RNE(A*(255/256) + 0.499))
                     = clip(A + [A<0], -128, 127) == trunc-clip(v)
    pass1 is the bottleneck (PSUM fp32 reads are 1x on every engine), so it
    is split across BOTH ScalarE and VectorE, and pass2 is split across
    VectorE (16-bit 2x mode) and GpSimd:
      ScalarE pass1 (fused single-rounding fma, validated exhaustively over
        every reachable q):  A = sat_i16(RNE(q*s + (zp - 0.5 + 2^-18)))
      VectorE pass1 (two fp32 roundings; (add, mult) op order so y is a
        full-mantissa product -- tie-free; validated exhaustively):
        A = sat_i16(RNE(fl(fl(q + b0) * s2))),
        b0 = fl((zp - 0.5)/s + 2^-8),  s2 = fl(s * (1 + 2^-22))
  - x^T columns are host-permuted so psum partition p owns output rows
    s = 8p+j: each partition's 8 rows form one contiguous 8 KiB DRAM run,
    keeping the output store near line rate
  - int8 results are staged in pool-aligned [128, 2048] SBUF tiles (offset
    slices of a bigger tile demote the VectorE op from 2x to 1x - measured)
"""

from contextlib import ExitStack
import numpy as np
import ml_dtypes

import concourse.bacc as bacc
import concourse.tile as tile
from concourse import mybir
from concourse.bass_utils import run_bass_kernel_spmd

AF = mybir.ActivationFunctionType
OP = mybir.AluOpType
BF16 = mybir.dt.bfloat16

N_CORES = 8
B, H, S, D = 8, 12, 1024, 64
HEADS_PER_CORE = B * H // N_CORES          # 12
N_PAIRS = HEADS_PER_CORE // 2              # 6
M_BLOCKS = S // 128                        # 8
T = 1024

# set by kernel() for test.py / bench.py to inspect
LAST_RESULTS = None
LAST_PREP = None

_NC_CACHE = {}

# --- engine-assignment knobs (absolute group index 0..47)
# Single-core HW loop-slope A/B (hw_ab*.py, method validated by reproducing
# the 99795ns baseline at 103253):
#   all-ACT pass1 (old):      103253 ns/iter
#   x14 DVE-p1, no GP:         96038 ns/iter
#   x12 DVE-p1, no GP:         beats x14g0 by ~6% in-run  <- shipped
#   x14 DVE-p1 + 14 GP pass2: 127053 ns/iter   (GP op in-kernel ~5.4us, not
#   14 GP pass2 only:         178714 ns/iter    the 1.7us it costs alone --
#                                               shared-SBUF-port contention)
# Absolute scale drifts ~25% between runs (p-state?); only within-run
# rankings are trusted.  TimelineSim mis-models GpSimd contention; trust HW.
DVE_P1_SET = frozenset(g for g in range(48) if g % 8 in (3, 6))
GP_P2_SET = frozenset()
# 4-slot psum mode ([128,1024] tiles, bufs=4): relaxes the 2-slot
# drain->refill chain but the +352+172-cycle per-instruction overhead on the
# halved drains outweighs it (sims 104-118k).  Keep 2-slot.
PSUM4 = False
DVE_H_OF_96 = 45        # of the 96 half-drains, how many go to VectorE
# pass1 column split: ACT drains [0:ACT_COLS), DVE drains [ACT_COLS:2048)
# concurrently.  None = whole-tile pass1 per DVE_P1_SET.  Splitting keeps the
# psum slot-hold short (ACT part only) so the drain->matmul->drain slot chain
# never stalls ACT, while giving DVE a tunable share of the 1x psum reads.
ACT_COLS = None
DVE_P1_HALVES = False   # DVE pass1 drains in two [128,1024] halves, each
                        # emitted right after its matmuls (sims worse: the
                        # early-emitted halves head-block the DVE FIFO)
DEFER_N = 2             # how many groups the VectorE pass2 trails its pass1
NH_SPLIT = 2            # rhs blocks per ji (N=512 moving operand; N=1024
                        # simmed slightly worse)
SPLIT_FIRST_LOAD = True


def _build_core_program(s_const: float, bias_a: float, c_b: float, d_b: float,
                        loop_iters: int | None = None):
    """One NeuronCore's program: 12 heads of [1024,64]@[64,1024] + requant.

    loop_iters: when set, wraps the whole body in a hardware For_i loop -
    used only for benchmarking (device time scales with the loop count so a
    slope isolates HW exec time from host/relay dispatch overhead).
    """
    nc = bacc.Bacc("TRN2", target_bir_lowering=False, debug=False)
    # head-pairs stacked on the partition axis
    d_xt = nc.dram_tensor("xt", [N_PAIRS, 128, S], BF16, kind="ExternalInput")
    d_yp = nc.dram_tensor("yp", [N_PAIRS, 128, T], BF16, kind="ExternalInput")
    d_o = nc.dram_tensor("o", [HEADS_PER_CORE, S, T], mybir.dt.int8,
                         kind="ExternalOutput")

    with tile.TileContext(nc) as tc:
        with ExitStack() as stk:
            if loop_iters is not None:
                # PE's body exceeds one IRAM block; hint the back-edge so the
                # benchmark loop doesn't pay a ~3-4 us ifetch per iteration
                # that single-shot execution would not pay.
                stk.enter_context(tc.For_i(0, loop_iters, 1,
                                           hint_engines=(mybir.EngineType.PE,)))
            _emit_body(nc, tc, d_xt, d_yp, d_o, s_const, bias_a, c_b, d_b)
    nc.compile()
    return nc


def _emit_body(nc, tc, d_xt, d_yp, d_o, s_const, bias_a, c_b, d_b):
    # VectorE one-op pass1 constants (validated in validate_requant.py over
    # every reachable q: 0 mismatches, max tie margin variant)
    s64 = np.float64(np.float32(s_const))
    zp64 = np.float64(np.float32(bias_a)) + 0.5 - 2.0 ** -18  # recover out_zp
    b0 = float(np.float32((zp64 - 0.5) / s64 + 2.0 ** -8))
    s2 = float(np.float32(s64 * (1.0 + 2.0 ** -22)))

    gidx = 0
    hidx = 0
    dve_h = 0
    pending = []
    ps_shape = [128, 1024] if PSUM4 else [128, 2048]
    ps_bufs = 4 if PSUM4 else 2
    with tc.tile_pool(name="xin", bufs=2) as xpool, \
         tc.tile_pool(name="yin", bufs=2) as ypool, \
         tc.tile_pool(name="aint", bufs=6) as apool, \
         tc.tile_pool(name="agp", bufs=4) as gpool, \
         tc.tile_pool(name="obuf", bufs=3) as opool, \
         tc.tile_pool(name="ps", bufs=ps_bufs, space="PSUM") as pspool:
        tiles = [None] * N_PAIRS
        xt0 = xpool.tile([128, S], BF16, tag="xt")
        yp0 = ypool.tile([128, T], BF16, tag="yp")
        if SPLIT_FIRST_LOAD:
            # single-shot prologue: land group-0's operands first so the
            # first matmuls (and ScalarE) start ~2 us earlier
            nc.sync.dma_start(xt0[:, 0:256], d_xt[0, :, 0:256])
            nc.sync.dma_start(yp0[:], d_yp[0, :, :])
            nc.sync.dma_start(xt0[:, 256:S], d_xt[0, :, 256:S])
        else:
            nc.sync.dma_start(xt0[:], d_xt[0, :, :])
            nc.sync.dma_start(yp0[:], d_yp[0, :, :])
        tiles[0] = (xt0, yp0)

        for pair in range(N_PAIRS):
            xt_t, yp_t = tiles[pair]
            if pair + 1 < N_PAIRS:
                # prefetch next pair's operands now so the loads sit ahead
                # of this pair's output stores in the SP HWDGE FIFO
                # (loads emitted at next pair's top would stall ~2.1 us/pair
                # behind the stores otherwise)
                xt_n = xpool.tile([128, S], BF16, tag="xt")
                yp_n = ypool.tile([128, T], BF16, tag="yp")
                nc.sync.dma_start(xt_n[:], d_xt[pair + 1, :, :])
                nc.sync.dma_start(yp_n[:], d_yp[pair + 1, :, :])
                tiles[pair + 1] = (xt_n, yp_n)

            ob = [[opool.tile([128, 2048], mybir.dt.int8,
                              tag=f"obs{jg}", name=f"ob_{pair}_{h2}_{jg}")
                   for jg in range(M_BLOCKS // 2)] for h2 in range(2)]

            # j-groups of 2 phases -> one [128, 2048] psum tile (4 banks);
            # two tiles ping-pong across all 8 banks while pass1 drains.
            for jg in range(M_BLOCKS // 2):
                for h2 in range(2):
                    g = gidx
                    gidx += 1
                    on_gp = g in GP_P2_SET
                    pool = gpool if on_gp else apool
                    a_t = pool.tile([128, 2048], mybir.dt.int16,
                                    tag="agp" if on_gp else "a")
                    ps = (None if PSUM4 else
                          pspool.tile([128, 2048], mybir.dt.float32,
                                      tag="ps"))
                    for ji in range(2):
                        j = jg * 2 + ji
                        lhsT = xt_t[64 * h2:64 * h2 + 64,
                                    j * 128:(j + 1) * 128]
                        if PSUM4:
                            psj = pspool.tile([128, 1024], mybir.dt.float32,
                                              tag="ps", name=f"psj_{g}_{ji}")
                        else:
                            psj = ps[:, ji * 1024:(ji + 1) * 1024]
                        for nh in range(NH_SPLIT):
                            nw = 1024 // NH_SPLIT
                            nc.tensor.matmul(
                                psj[:, nh * nw:(nh + 1) * nw],
                                lhsT,
                                yp_t[64 * h2:64 * h2 + 64,
                                     nh * nw:(nh + 1) * nw],
                                start=True, stop=True,
                                tile_position=(64 * h2, 0),
                            )
                        if PSUM4:
                            # drain this half now; slot-chain relaxed by the
                            # 4-buffer rotation.  Engine via Bresenham ratio.
                            a_dst = a_t[:, ji * 1024:(ji + 1) * 1024]
                            want_dve = ((hidx + 1) * DVE_H_OF_96) // 96 \
                                > dve_h
                            hidx += 1
                            if want_dve:
                                dve_h += 1
                                nc.vector.tensor_scalar(a_dst, psj[:],
                                                        b0, s2,
                                                        OP.add, OP.mult)
                            else:
                                nc.scalar.activation(a_dst, psj[:], AF.Copy,
                                                     bias=bias_a,
                                                     scale=s_const)
                    if PSUM4:
                        pass
                    elif ACT_COLS is not None:
                        # concurrent split drain: both engines read the psum
                        # tile at once (separate read ports); both int16
                        # encodings are the same exact floor(v)
                        ca = ACT_COLS
                        nc.scalar.activation(a_t[:, 0:ca], ps[:, 0:ca],
                                             AF.Copy,
                                             bias=bias_a, scale=s_const)
                        nc.vector.tensor_scalar(a_t[:, ca:2048],
                                                ps[:, ca:2048],
                                                b0, s2, OP.add, OP.mult)
                    elif g in DVE_P1_SET:
                        # VectorE one-op pass1 (frees ScalarE; emitted now so
                        # the psum slot drains promptly)
                        nc.vector.tensor_scalar(a_t[:], ps[:],
                                                b0, s2, OP.add, OP.mult)
                    else:
                        nc.scalar.activation(a_t[:], ps[:], AF.Copy,
                                             bias=bias_a, scale=s_const)
                    if on_gp:
                        nc.gpsimd.tensor_scalar(ob[h2][jg][:], a_t[:],
                                                c_b, d_b, OP.mult, OP.add)
                    else:
                        def fmap(a_t=a_t, ob_t=ob[h2][jg]):
                            nc.vector.tensor_scalar(ob_t[:], a_t[:],
                                                    c_b, d_b,
                                                    OP.mult, OP.add)
                        pending.append(fmap)
                    while len(pending) > DEFER_N:
                        pending.pop(0)()
            # flush this pair's remaining pass2 ops, then batched output DMAs
            while pending:
                pending.pop(0)()
            for h2 in range(2):
                dst = d_o[2 * pair + h2, :, :].rearrange(
                    "(p j) t -> p (j t)", j=M_BLOCKS)
                for jg in range(M_BLOCKS // 2):
                    nc.sync.dma_start(dst[:, jg * 2048:(jg + 1) * 2048],
                                      ob[h2][jg][:])


def kernel(x, y, alpha, a_zp, b_zp, out_zp, o_alpha):
    global LAST_RESULTS, LAST_PREP
    x = np.asarray(x)
    y = np.asarray(y)
    s_const = float(np.float32(np.float32(alpha) / np.float32(o_alpha)))
    bias_a = float(np.float64(np.float32(out_zp)) - 0.5 + 2.0 ** -18)
    c_b = float(np.float32(255.0 / 256.0))
    d_b = float(np.float32(0.499))

    # ---- host-side shard + dequant prep (exact in bf16) ----
    xf = x.reshape(B * H, S, D).astype(np.float32) - np.float32(a_zp)
    yf = y.reshape(B * H, D, T).astype(np.float32) - np.float32(b_zp)
    # lhsT layout: [head, D, S], head-pairs stacked to 128 partitions.
    # S-columns permuted to c = j*128 + p  <->  s = 8p + j so each psum
    # partition owns 8 consecutive output rows (8 KiB DMA runs).
    xt = np.ascontiguousarray(xf.transpose(0, 2, 1)).astype(ml_dtypes.bfloat16)
    xt = np.ascontiguousarray(
        xt.reshape(B * H, D, S // 8, 8).transpose(0, 1, 3, 2)).reshape(
        B * H, D, S)
    yp = yf.astype(ml_dtypes.bfloat16)
    xt = xt.reshape(N_CORES, N_PAIRS, 128, S)
    yp = yp.reshape(N_CORES, N_PAIRS, 128, T)

    key = (s_const, bias_a, c_b, d_b)
    if key not in _NC_CACHE:
        _NC_CACHE[key] = _build_core_program(*key)
    nc = _NC_CACHE[key]

    in_maps = [{"xt": xt[c], "yp": yp[c]} for c in range(N_CORES)]
    LAST_PREP = (key, in_maps)
    res = run_bass_kernel_spmd(nc, in_maps, core_ids=list(range(N_CORES)))
    LAST_RESULTS = res

    out = np.stack([res.results[c]["o"] for c in range(N_CORES)])
    return out.reshape(B, H, S, T)


if __name__ == "__main__":
    rng = np.random.default_rng(0)
    x = rng.integers(-128, 128, size=(B, H, S, D)).astype(np.int32)
    y = rng.integers(-128, 128, size=(B, H, D, T)).astype(np.int32)
    out = kernel(x=x, y=y, alpha=np.float32(0.000234), a_zp=np.float32(3.0),
                 b_zp=np.float32(-5.0), out_zp=np.float32(2.0),
                 o_alpha=np.float32(0.0625))
    print("kernel output", out.shape, out.dtype)



# revision 3
# speedup vs baseline: 15.1015x; 15.1015x over previous
"""Trainium2 Bass kernel for nn_BMMS8TS8NS8T: batched int8-valued GEMM with
dequant/requant, sharded head-parallel across 8 NeuronCores.

Reference semantics (jax CPU, fp32):
    a = x.float() - a_zp          # [B,H,S,D]  int8-valued
    b = y.float() - b_zp          # [B,H,D,T]
    q = a @ b                     # exact integers (|q| <= 64*131*132 < 2^24)
    v = fl(fl(q * s) + out_zp),   s = fl(alpha / o_alpha)
    out = trunc(clip(v, -128, 127)).astype(int8)   # trunc toward zero

Device strategy per core (12 heads = (B*H)/8, head parallel, no cross-core
communication):
  - host pre-dequantizes inputs to bf16 (exact: all values are integers with
    |v| <= 132 < 256, exactly representable in bf16) and pre-transposes /
    permutes x so the stationary matmul operand needs no on-device transpose
  - TensorE: K=64 matmuls, two heads packed in the 128-row PE array via row
    tiling (tile_position (0,0)/(64,0)); fp32 PSUM accumulation is exact
  - requantization is a two-pass scheme (exact trunc toward zero cannot be a
    single affine op + RNE convert: the trunc bin at 0 is double-width):
      pass1:  A_i16 = floor(v)   (clipped by i16 saturation far out)
      pass2:  out_i8 = sat_i8(RNE(A*(255/256) + 0.499))
                     = clip(A + [A<0], -128, 127) == trunc-clip(v)
    pass1 is the bottleneck (PSUM fp32 reads are 1x on every engine), so it
    is split across BOTH ScalarE and VectorE, and pass2 is split across
    VectorE (16-bit 2x mode) and GpSimd:
      ScalarE pass1 (fused single-rounding fma, validated exhaustively over
        every reachable q):  A = sat_i16(RNE(q*s + (zp - 0.5 + 2^-18)))
      VectorE pass1 (two fp32 roundings; (add, mult) op order so y is a
        full-mantissa product -- tie-free; validated exhaustively):
        A = sat_i16(RNE(fl(fl(q + b0) * s2))),
        b0 = fl((zp - 0.5)/s + 2^-8),  s2 = fl(s * (1 + 2^-22))
  - x^T columns are host-permuted so psum partition p owns output rows
    s = 8p+j: each partition's 8 rows form one contiguous 8 KiB DRAM run,
    keeping the output store near line rate
  - int8 results are staged in pool-aligned [128, 2048] SBUF tiles (offset
    slices of a bigger tile demote the VectorE op from 2x to 1x - measured)
"""

from contextlib import ExitStack
import numpy as np
import ml_dtypes

import concourse.bacc as bacc
import concourse.tile as tile
from concourse import mybir
from concourse.bass_utils import run_bass_kernel_spmd

AF = mybir.ActivationFunctionType
OP = mybir.AluOpType
BF16 = mybir.dt.bfloat16

N_CORES = 8
B, H, S, D = 8, 12, 1024, 64
HEADS_PER_CORE = B * H // N_CORES          # 12
N_PAIRS = HEADS_PER_CORE // 2              # 6
M_BLOCKS = S // 128                        # 8
T = 1024

# set by kernel() for test.py / bench.py to inspect
LAST_RESULTS = None
LAST_PREP = None

_NC_CACHE = {}

# --- engine-assignment knobs (absolute group index 0..47)
# Single-core HW loop-slope A/B (hw_ab*.py, method validated by reproducing
# the 99795ns baseline at 103253):
#   all-ACT pass1 (old):      103253 ns/iter
#   x14 DVE-p1, no GP:         96038 ns/iter
#   x12 DVE-p1, no GP:         beats x14g0 by ~6% in-run  <- shipped
#   x14 DVE-p1 + 14 GP pass2: 127053 ns/iter   (GP op in-kernel ~5.4us, not
#   14 GP pass2 only:         178714 ns/iter    the 1.7us it costs alone --
#                                               shared-SBUF-port contention)
# Absolute scale drifts ~25% between runs (p-state?); only within-run
# rankings are trusted.  TimelineSim mis-models GpSimd contention; trust HW.
DVE_P1_SET = frozenset(g for g in range(48) if g % 8 in (3, 6))
GP_P2_SET = frozenset()
# 4-slot psum mode ([128,1024] tiles, bufs=4): relaxes the 2-slot
# drain->refill chain but the +352+172-cycle per-instruction overhead on the
# halved drains outweighs it (sims 104-118k).  Keep 2-slot.
PSUM4 = False
DVE_H_OF_96 = 45        # of the 96 half-drains, how many go to VectorE
# pass1 column split: ACT drains [0:ACT_COLS), DVE drains [ACT_COLS:2048)
# concurrently.  None = whole-tile pass1 per DVE_P1_SET.  Splitting keeps the
# psum slot-hold short (ACT part only) so the drain->matmul->drain slot chain
# never stalls ACT, while giving DVE a tunable share of the 1x psum reads.
ACT_COLS = None
DVE_P1_HALVES = False   # DVE pass1 drains in two [128,1024] halves, each
                        # emitted right after its matmuls (sims worse: the
                        # early-emitted halves head-block the DVE FIFO)
DEFER_N = 2             # how many groups the VectorE pass2 trails its pass1
NH_SPLIT = 2            # rhs blocks per ji (N=512 moving operand; N=1024
                        # simmed slightly worse)
SPLIT_FIRST_LOAD = True


def _build_core_program(s_const: float, bias_a: float, c_b: float, d_b: float,
                        loop_iters: int | None = None,
                        bench_io: bool = False):
    """One NeuronCore's program: 12 heads of [1024,64]@[64,1024] + requant.

    loop_iters: when set, wraps the whole body in a hardware For_i loop -
    used only for benchmarking (device time scales with the loop count so a
    slope isolates HW exec time from host/relay dispatch overhead).

    bench_io: all big DRAM tensors become Internal (device-resident garbage,
    never shipped over the axon relay) and a tiny [1,16] ExternalOutput is
    added so PJRT has something to return.  Timing is data-independent, so
    the loop slope is unaffected; per-call payload drops from ~15 MB/core to
    16 bytes.
    """
    nc = bacc.Bacc("TRN2", target_bir_lowering=False, debug=False)
    io_kind_in = "Internal" if bench_io else "ExternalInput"
    io_kind_out = "Internal" if bench_io else "ExternalOutput"
    # head-pairs stacked on the partition axis
    d_xt = nc.dram_tensor("xt", [N_PAIRS, 128, S], BF16, kind=io_kind_in)
    d_yp = nc.dram_tensor("yp", [N_PAIRS, 128, T], BF16, kind=io_kind_in)
    d_o = nc.dram_tensor("o", [HEADS_PER_CORE, S, T], mybir.dt.int8,
                         kind=io_kind_out)
    d_tick = (nc.dram_tensor("tick", [1, 16], mybir.dt.int8,
                             kind="ExternalOutput") if bench_io else None)

    with tile.TileContext(nc) as tc:
        with ExitStack() as stk:
            if loop_iters is not None:
                # PE's body exceeds one IRAM block; hint the back-edge so the
                # benchmark loop doesn't pay a ~3-4 us ifetch per iteration
                # that single-shot execution would not pay.
                stk.enter_context(tc.For_i(0, loop_iters, 1,
                                           hint_engines=(mybir.EngineType.PE,)))
            _emit_body(nc, tc, d_xt, d_yp, d_o, s_const, bias_a, c_b, d_b)
        if d_tick is not None:
            with tc.tile_pool(name="tick", bufs=1) as tkpool:
                tk = tkpool.tile([1, 16], mybir.dt.int8, tag="tick")
                nc.vector.memset(tk[:], 0)
                nc.sync.dma_start(d_tick[:, :], tk[:])
    nc.compile()
    return nc


def _emit_body(nc, tc, d_xt, d_yp, d_o, s_const, bias_a, c_b, d_b):
    # VectorE one-op pass1 constants (validated in validate_requant.py over
    # every reachable q: 0 mismatches, max tie margin variant)
    s64 = np.float64(np.float32(s_const))
    zp64 = np.float64(np.float32(bias_a)) + 0.5 - 2.0 ** -18  # recover out_zp
    b0 = float(np.float32((zp64 - 0.5) / s64 + 2.0 ** -8))
    s2 = float(np.float32(s64 * (1.0 + 2.0 ** -22)))

    gidx = 0
    hidx = 0
    dve_h = 0
    pending = []
    ps_shape = [128, 1024] if PSUM4 else [128, 2048]
    ps_bufs = 4 if PSUM4 else 2
    with tc.tile_pool(name="xin", bufs=2) as xpool, \
         tc.tile_pool(name="yin", bufs=2) as ypool, \
         tc.tile_pool(name="aint", bufs=6) as apool, \
         tc.tile_pool(name="agp", bufs=4) as gpool, \
         tc.tile_pool(name="obuf", bufs=3) as opool, \
         tc.tile_pool(name="ps", bufs=ps_bufs, space="PSUM") as pspool:
        tiles = [None] * N_PAIRS
        xt0 = xpool.tile([128, S], BF16, tag="xt")
        yp0 = ypool.tile([128, T], BF16, tag="yp")
        if SPLIT_FIRST_LOAD:
            # single-shot prologue: land group-0's operands first so the
            # first matmuls (and ScalarE) start ~2 us earlier
            nc.sync.dma_start(xt0[:, 0:256], d_xt[0, :, 0:256])
            nc.sync.dma_start(yp0[:], d_yp[0, :, :])
            nc.sync.dma_start(xt0[:, 256:S], d_xt[0, :, 256:S])
        else:
            nc.sync.dma_start(xt0[:], d_xt[0, :, :])
            nc.sync.dma_start(yp0[:], d_yp[0, :, :])
        tiles[0] = (xt0, yp0)

        for pair in range(N_PAIRS):
            xt_t, yp_t = tiles[pair]
            if pair + 1 < N_PAIRS:
                # prefetch next pair's operands now so the loads sit ahead
                # of this pair's output stores in the SP HWDGE FIFO
                # (loads emitted at next pair's top would stall ~2.1 us/pair
                # behind the stores otherwise)
                xt_n = xpool.tile([128, S], BF16, tag="xt")
                yp_n = ypool.tile([128, T], BF16, tag="yp")
                nc.sync.dma_start(xt_n[:], d_xt[pair + 1, :, :])
                nc.sync.dma_start(yp_n[:], d_yp[pair + 1, :, :])
                tiles[pair + 1] = (xt_n, yp_n)

            ob = [[opool.tile([128, 2048], mybir.dt.int8,
                              tag=f"obs{jg}", name=f"ob_{pair}_{h2}_{jg}")
                   for jg in range(M_BLOCKS // 2)] for h2 in range(2)]

            # j-groups of 2 phases -> one [128, 2048] psum tile (4 banks);
            # two tiles ping-pong across all 8 banks while pass1 drains.
            for jg in range(M_BLOCKS // 2):
                for h2 in range(2):
                    g = gidx
                    gidx += 1
                    on_gp = g in GP_P2_SET
                    pool = gpool if on_gp else apool
                    a_t = pool.tile([128, 2048], mybir.dt.int16,
                                    tag="agp" if on_gp else "a")
                    ps = (None if PSUM4 else
                          pspool.tile([128, 2048], mybir.dt.float32,
                                      tag="ps"))
                    for ji in range(2):
                        j = jg * 2 + ji
                        lhsT = xt_t[64 * h2:64 * h2 + 64,
                                    j * 128:(j + 1) * 128]
                        if PSUM4:
                            psj = pspool.tile([128, 1024], mybir.dt.float32,
                                              tag="ps", name=f"psj_{g}_{ji}")
                        else:
                            psj = ps[:, ji * 1024:(ji + 1) * 1024]
                        for nh in range(NH_SPLIT):
                            nw = 1024 // NH_SPLIT
                            nc.tensor.matmul(
                                psj[:, nh * nw:(nh + 1) * nw],
                                lhsT,
                                yp_t[64 * h2:64 * h2 + 64,
                                     nh * nw:(nh + 1) * nw],
                                start=True, stop=True,
                                tile_position=(64 * h2, 0),
                            )
                        if PSUM4:
                            # drain this half now; slot-chain relaxed by the
                            # 4-buffer rotation.  Engine via Bresenham ratio.
                            a_dst = a_t[:, ji * 1024:(ji + 1) * 1024]
                            want_dve = ((hidx + 1) * DVE_H_OF_96) // 96 \
                                > dve_h
                            hidx += 1
                            if want_dve:
                                dve_h += 1
                                nc.vector.tensor_scalar(a_dst, psj[:],
                                                        b0, s2,
                                                        OP.add, OP.mult)
                            else:
                                nc.scalar.activation(a_dst, psj[:], AF.Copy,
                                                     bias=bias_a,
                                                     scale=s_const)
                    if PSUM4:
                        pass
                    elif ACT_COLS is not None:
                        # concurrent split drain: both engines read the psum
                        # tile at once (separate read ports); both int16
                        # encodings are the same exact floor(v)
                        ca = ACT_COLS
                        nc.scalar.activation(a_t[:, 0:ca], ps[:, 0:ca],
                                             AF.Copy,
                                             bias=bias_a, scale=s_const)
                        nc.vector.tensor_scalar(a_t[:, ca:2048],
                                                ps[:, ca:2048],
                                                b0, s2, OP.add, OP.mult)
                    elif g in DVE_P1_SET:
                        # VectorE one-op pass1 (frees ScalarE; emitted now so
                        # the psum slot drains promptly)
                        nc.vector.tensor_scalar(a_t[:], ps[:],
                                                b0, s2, OP.add, OP.mult)
                    else:
                        nc.scalar.activation(a_t[:], ps[:], AF.Copy,
                                             bias=bias_a, scale=s_const)
                    if on_gp:
                        nc.gpsimd.tensor_scalar(ob[h2][jg][:], a_t[:],
                                                c_b, d_b, OP.mult, OP.add)
                    else:
                        def fmap(a_t=a_t, ob_t=ob[h2][jg]):
                            nc.vector.tensor_scalar(ob_t[:], a_t[:],
                                                    c_b, d_b,
                                                    OP.mult, OP.add)
                        pending.append(fmap)
                    while len(pending) > DEFER_N:
                        pending.pop(0)()
            # flush this pair's remaining pass2 ops, then batched output DMAs
            while pending:
                pending.pop(0)()
            for h2 in range(2):
                dst = d_o[2 * pair + h2, :, :].rearrange(
                    "(p j) t -> p (j t)", j=M_BLOCKS)
                for jg in range(M_BLOCKS // 2):
                    nc.sync.dma_start(dst[:, jg * 2048:(jg + 1) * 2048],
                                      ob[h2][jg][:])


def default_key():
    """Requant constants for the reference problem's quantization params -
    used by bench.py when kernel() hasn't run in this process."""
    s_const = float(np.float32(np.float32(0.000234) / np.float32(0.0625)))
    bias_a = float(np.float64(np.float32(2.0)) - 0.5 + 2.0 ** -18)
    c_b = float(np.float32(255.0 / 256.0))
    d_b = float(np.float32(0.499))
    return (s_const, bias_a, c_b, d_b)


def kernel(x, y, alpha, a_zp, b_zp, out_zp, o_alpha):
    global LAST_RESULTS, LAST_PREP
    x = np.asarray(x)
    y = np.asarray(y)
    s_const = float(np.float32(np.float32(alpha) / np.float32(o_alpha)))
    bias_a = float(np.float64(np.float32(out_zp)) - 0.5 + 2.0 ** -18)
    c_b = float(np.float32(255.0 / 256.0))
    d_b = float(np.float32(0.499))

    # ---- host-side shard + dequant prep (exact in bf16) ----
    xf = x.reshape(B * H, S, D).astype(np.float32) - np.float32(a_zp)
    yf = y.reshape(B * H, D, T).astype(np.float32) - np.float32(b_zp)
    # lhsT layout: [head, D, S], head-pairs stacked to 128 partitions.
    # S-columns permuted to c = j*128 + p  <->  s = 8p + j so each psum
    # partition owns 8 consecutive output rows (8 KiB DMA runs).
    xt = np.ascontiguousarray(xf.transpose(0, 2, 1)).astype(ml_dtypes.bfloat16)
    xt = np.ascontiguousarray(
        xt.reshape(B * H, D, S // 8, 8).transpose(0, 1, 3, 2)).reshape(
        B * H, D, S)
    yp = yf.astype(ml_dtypes.bfloat16)
    xt = xt.reshape(N_CORES, N_PAIRS, 128, S)
    yp = yp.reshape(N_CORES, N_PAIRS, 128, T)

    key = (s_const, bias_a, c_b, d_b)
    if key not in _NC_CACHE:
        _NC_CACHE[key] = _build_core_program(*key)
    nc = _NC_CACHE[key]

    in_maps = [{"xt": xt[c], "yp": yp[c]} for c in range(N_CORES)]
    LAST_PREP = (key, in_maps)
    res = run_bass_kernel_spmd(nc, in_maps, core_ids=list(range(N_CORES)))
    LAST_RESULTS = res

    out = np.stack([res.results[c]["o"] for c in range(N_CORES)])
    return out.reshape(B, H, S, T)


if __name__ == "__main__":
    rng = np.random.default_rng(0)
    x = rng.integers(-128, 128, size=(B, H, S, D)).astype(np.int32)
    y = rng.integers(-128, 128, size=(B, H, D, T)).astype(np.int32)
    out = kernel(x=x, y=y, alpha=np.float32(0.000234), a_zp=np.float32(3.0),
                 b_zp=np.float32(-5.0), out_zp=np.float32(2.0),
                 o_alpha=np.float32(0.0625))
    print("kernel output", out.shape, out.dtype)



# revision 5
# speedup vs baseline: 18.5621x; 1.2292x over previous
"""Trainium2 Bass kernel for nn_BMMS8TS8NS8T: batched int8-valued GEMM with
dequant/requant, sharded head-parallel across 8 NeuronCores.

Reference semantics (jax CPU, fp32):
    a = x.float() - a_zp          # [B,H,S,D]  int8-valued
    b = y.float() - b_zp          # [B,H,D,T]
    q = a @ b                     # exact integers (|q| <= 64*131*132 < 2^24)
    v = fl(fl(q * s) + out_zp),   s = fl(alpha / o_alpha)
    out = trunc(clip(v, -128, 127)).astype(int8)   # trunc toward zero

Device strategy per core (12 heads = (B*H)/8, head parallel, no cross-core
communication):
  - host pre-dequantizes inputs to bf16 (exact: all values are integers with
    |v| <= 132 < 256, exactly representable in bf16) and pre-transposes /
    permutes x so the stationary matmul operand needs no on-device transpose
  - TensorE: K=64 matmuls, two heads packed in the 128-row PE array via row
    tiling (tile_position (0,0)/(64,0)); fp32 PSUM accumulation is exact
  - requantization is a two-pass scheme (exact trunc toward zero cannot be a
    single affine op + RNE convert: the trunc bin at 0 is double-width):
      pass1:  A_i16 = floor(v)   (clipped by i16 saturation far out)
      pass2:  out_i8 = sat_i8(RNE(A*(255/256) + 0.499))
                     = clip(A + [A<0], -128, 127) == trunc-clip(v)
    pass1 is the bottleneck (PSUM fp32 reads are 1x on every engine), so it
    is split across BOTH ScalarE and VectorE, and pass2 is split across
    VectorE (16-bit 2x mode) and GpSimd:
      ScalarE pass1 (fused single-rounding fma, validated exhaustively over
        every reachable q):  A = sat_i16(RNE(q*s + (zp - 0.5 + 2^-18)))
      VectorE pass1 (two fp32 roundings; (add, mult) op order so y is a
        full-mantissa product -- tie-free; validated exhaustively):
        A = sat_i16(RNE(fl(fl(q + b0) * s2))),
        b0 = fl((zp - 0.5)/s + 2^-8),  s2 = fl(s * (1 + 2^-22))
  - x^T columns are host-permuted so psum partition p owns output rows
    s = 8p+j: each partition's 8 rows form one contiguous 8 KiB DRAM run,
    keeping the output store near line rate
  - int8 results are staged in pool-aligned [128, 2048] SBUF tiles (offset
    slices of a bigger tile demote the VectorE op from 2x to 1x - measured)
"""

from contextlib import ExitStack
import numpy as np
import ml_dtypes

import concourse.bacc as bacc
import concourse.tile as tile
from concourse import mybir
from concourse.bass_utils import run_bass_kernel_spmd

AF = mybir.ActivationFunctionType
OP = mybir.AluOpType
BF16 = mybir.dt.bfloat16

N_CORES = 8
B, H, S, D = 8, 12, 1024, 64
HEADS_PER_CORE = B * H // N_CORES          # 12
N_PAIRS = HEADS_PER_CORE // 2              # 6
M_BLOCKS = S // 128                        # 8
T = 1024

# set by kernel() for test.py / bench.py to inspect
LAST_RESULTS = None
LAST_PREP = None

_NC_CACHE = {}

# --- engine-assignment knobs (v2 structure)
# v2: 4-quad psum rotation ([128,1024] tiles, bufs=4) decouples matmul fills
# from drains (the old 2x[128,2048] ping-pong was period-bound: a tile's
# refill waited on its own ~2.1us drain, capping throughput at
# (fill+drain)/2 per group ~= 1.5us -> ~71us/iter).  Engine balance:
#   pass1 (psum fp32 -> int16, 1x on every engine; Pool has NO psum port):
#     ACT 1.2GHz (+ ~0.39us/op overhead, non-pipelined), DVE 0.96GHz.
#   pass2 (int16 -> int8): DVE scalar_tensor_tensor (A<0)+A at 16-bit
#     2x_1P (never grabs the DVE/Pool shared SBUF port pair), Pool
#     tensor_scalar (c_b,d_b) ~1x.
ACT_P1_OF_96 = 54       # of the 96 quad-drains, how many go to ScalarE
DVE_P2_OF_48 = 11       # of the 48 pass2 ops, how many go to VectorE (STT)
DEFER_N = 2             # how many groups the pass2 trails its pass1
NH_SPLIT = 2            # rhs blocks per ji (N=512 moving operand)
SPLIT_FIRST_LOAD = True


def _build_core_program(s_const: float, bias_a: float, c_b: float, d_b: float,
                        loop_iters: int | None = None,
                        bench_io: bool = False):
    """One NeuronCore's program: 12 heads of [1024,64]@[64,1024] + requant.

    loop_iters: when set, wraps the whole body in a hardware For_i loop -
    used only for benchmarking (device time scales with the loop count so a
    slope isolates HW exec time from host/relay dispatch overhead).

    bench_io: all big DRAM tensors become Internal (device-resident garbage,
    never shipped over the axon relay) and a tiny [1,16] ExternalOutput is
    added so PJRT has something to return.  Timing is data-independent, so
    the loop slope is unaffected; per-call payload drops from ~15 MB/core to
    16 bytes.
    """
    nc = bacc.Bacc("TRN2", target_bir_lowering=False, debug=False)
    io_kind_in = "Internal" if bench_io else "ExternalInput"
    io_kind_out = "Internal" if bench_io else "ExternalOutput"
    # head-pairs stacked on the partition axis
    d_xt = nc.dram_tensor("xt", [N_PAIRS, 128, S], BF16, kind=io_kind_in)
    d_yp = nc.dram_tensor("yp", [N_PAIRS, 128, T], BF16, kind=io_kind_in)
    d_o = nc.dram_tensor("o", [HEADS_PER_CORE, S, T], mybir.dt.int8,
                         kind=io_kind_out)
    d_tick = (nc.dram_tensor("tick", [1, 16], mybir.dt.int8,
                             kind="ExternalOutput") if bench_io else None)

    with tile.TileContext(nc) as tc:
        with ExitStack() as stk:
            if loop_iters is not None:
                # PE's body exceeds one IRAM block; hint the back-edge so the
                # benchmark loop doesn't pay a ~3-4 us ifetch per iteration
                # that single-shot execution would not pay.
                stk.enter_context(tc.For_i(0, loop_iters, 1,
                                           hint_engines=(mybir.EngineType.PE,)))
            _emit_body(nc, tc, d_xt, d_yp, d_o, s_const, bias_a, c_b, d_b)
        if d_tick is not None:
            with tc.tile_pool(name="tick", bufs=1) as tkpool:
                tk = tkpool.tile([1, 16], mybir.dt.int8, tag="tick")
                nc.vector.memset(tk[:], 0)
                nc.sync.dma_start(d_tick[:, :], tk[:])
    nc.compile()
    return nc


def _emit_body(nc, tc, d_xt, d_yp, d_o, s_const, bias_a, c_b, d_b):
    # VectorE one-op pass1 constants (validated in validate_requant.py over
    # every reachable q: 0 mismatches, max tie margin variant)
    s64 = np.float64(np.float32(s_const))
    zp64 = np.float64(np.float32(bias_a)) + 0.5 - 2.0 ** -18  # recover out_zp
    b0 = float(np.float32((zp64 - 0.5) / s64 + 2.0 ** -8))
    s2 = float(np.float32(s64 * (1.0 + 2.0 ** -22)))

    hidx = 0            # quad-drain counter (0..95)
    act_h = 0
    p2idx = 0           # pass2 counter (0..47)
    dve_p2 = 0
    pending = []
    with tc.tile_pool(name="xin", bufs=2) as xpool, \
         tc.tile_pool(name="yin", bufs=2) as ypool, \
         tc.tile_pool(name="aint", bufs=6) as apool, \
         tc.tile_pool(name="obuf", bufs=4) as opool, \
         tc.tile_pool(name="ps", bufs=4, space="PSUM") as pspool:
        tiles = [None] * N_PAIRS
        xt0 = xpool.tile([128, S], BF16, tag="xt")
        yp0 = ypool.tile([128, T], BF16, tag="yp")
        if SPLIT_FIRST_LOAD:
            # single-shot prologue: land group-0's operands first so the
            # first matmuls (and ScalarE) start ~2 us earlier
            nc.sync.dma_start(xt0[:, 0:256], d_xt[0, :, 0:256])
            nc.sync.dma_start(yp0[:], d_yp[0, :, :])
            nc.sync.dma_start(xt0[:, 256:S], d_xt[0, :, 256:S])
        else:
            nc.sync.dma_start(xt0[:], d_xt[0, :, :])
            nc.sync.dma_start(yp0[:], d_yp[0, :, :])
        tiles[0] = (xt0, yp0)

        for pair in range(N_PAIRS):
            xt_t, yp_t = tiles[pair]
            if pair + 1 < N_PAIRS:
                # prefetch next pair's operands now so the loads sit ahead
                # of this pair's output stores in the SP HWDGE FIFO
                xt_n = xpool.tile([128, S], BF16, tag="xt")
                yp_n = ypool.tile([128, T], BF16, tag="yp")
                nc.sync.dma_start(xt_n[:], d_xt[pair + 1, :, :])
                nc.sync.dma_start(yp_n[:], d_yp[pair + 1, :, :])
                tiles[pair + 1] = (xt_n, yp_n)

            ob = [[opool.tile([128, 2048], mybir.dt.int8,
                              tag=f"obs{jg}", name=f"ob_{pair}_{h2}_{jg}")
                   for jg in range(M_BLOCKS // 2)] for h2 in range(2)]

            for jg in range(M_BLOCKS // 2):
                for h2 in range(2):
                    a_t = apool.tile([128, 2048], mybir.dt.int16, tag="a")
                    for ji in range(2):
                        j = jg * 2 + ji
                        lhsT = xt_t[64 * h2:64 * h2 + 64,
                                    j * 128:(j + 1) * 128]
                        psj = pspool.tile([128, 1024], mybir.dt.float32,
                                          tag="ps")
                        for nh in range(NH_SPLIT):
                            nw = 1024 // NH_SPLIT
                            nc.tensor.matmul(
                                psj[:, nh * nw:(nh + 1) * nw],
                                lhsT,
                                yp_t[64 * h2:64 * h2 + 64,
                                     nh * nw:(nh + 1) * nw],
                                start=True, stop=True,
                                tile_position=(64 * h2, 0),
                            )
                        # drain this quad now; 4-buffer rotation keeps the
                        # fills decoupled.  Engine via Bresenham ratio.
                        a_dst = a_t[:, ji * 1024:(ji + 1) * 1024]
                        want_act = ((hidx + 1) * ACT_P1_OF_96) // 96 > act_h
                        hidx += 1
                        if want_act:
                            act_h += 1
                            nc.scalar.activation(a_dst, psj[:], AF.Copy,
                                                 bias=bias_a, scale=s_const)
                        else:
                            nc.vector.tensor_scalar(a_dst, psj[:],
                                                    b0, s2,
                                                    OP.add, OP.mult)
                    # pass2: deferred so it trails the drains by DEFER_N
                    # groups (keeps the engine FIFOs from head-blocking)
                    want_dve_p2 = ((p2idx + 1) * DVE_P2_OF_48) // 48 > dve_p2
                    p2idx += 1
                    if want_dve_p2:
                        dve_p2 += 1

                        def fmap(a_t=a_t, ob_t=ob[h2][jg]):
                            # exact: (A<0)+A == trunc correction; fp32 ALU,
                            # saturating RNE convert to int8.  STT form
                            # stays in 2x_1P (never locks Pool out of the
                            # shared SBUF port pair).
                            nc.vector.scalar_tensor_tensor(
                                ob_t[:], a_t[:], 0.0, a_t[:],
                                OP.is_lt, OP.add)
                    else:

                        def fmap(a_t=a_t, ob_t=ob[h2][jg]):
                            nc.gpsimd.tensor_scalar(ob_t[:], a_t[:],
                                                    c_b, d_b,
                                                    OP.mult, OP.add)
                    pending.append(fmap)
                    while len(pending) > DEFER_N:
                        pending.pop(0)()
            # flush this pair's remaining pass2 ops, then batched output DMAs
            while pending:
                pending.pop(0)()
            for h2 in range(2):
                dst = d_o[2 * pair + h2, :, :].rearrange(
                    "(p j) t -> p (j t)", j=M_BLOCKS)
                for jg in range(M_BLOCKS // 2):
                    nc.sync.dma_start(dst[:, jg * 2048:(jg + 1) * 2048],
                                      ob[h2][jg][:])


def default_key():
    """Requant constants for the reference problem's quantization params -
    used by bench.py when kernel() hasn't run in this process."""
    s_const = float(np.float32(np.float32(0.000234) / np.float32(0.0625)))
    bias_a = float(np.float64(np.float32(2.0)) - 0.5 + 2.0 ** -18)
    c_b = float(np.float32(255.0 / 256.0))
    d_b = float(np.float32(0.499))
    return (s_const, bias_a, c_b, d_b)


def kernel(x, y, alpha, a_zp, b_zp, out_zp, o_alpha):
    global LAST_RESULTS, LAST_PREP
    x = np.asarray(x)
    y = np.asarray(y)
    s_const = float(np.float32(np.float32(alpha) / np.float32(o_alpha)))
    bias_a = float(np.float64(np.float32(out_zp)) - 0.5 + 2.0 ** -18)
    c_b = float(np.float32(255.0 / 256.0))
    d_b = float(np.float32(0.499))

    # ---- host-side shard + dequant prep (exact in bf16) ----
    xf = x.reshape(B * H, S, D).astype(np.float32) - np.float32(a_zp)
    yf = y.reshape(B * H, D, T).astype(np.float32) - np.float32(b_zp)
    # lhsT layout: [head, D, S], head-pairs stacked to 128 partitions.
    # S-columns permuted to c = j*128 + p  <->  s = 8p + j so each psum
    # partition owns 8 consecutive output rows (8 KiB DMA runs).
    xt = np.ascontiguousarray(xf.transpose(0, 2, 1)).astype(ml_dtypes.bfloat16)
    xt = np.ascontiguousarray(
        xt.reshape(B * H, D, S // 8, 8).transpose(0, 1, 3, 2)).reshape(
        B * H, D, S)
    yp = yf.astype(ml_dtypes.bfloat16)
    xt = xt.reshape(N_CORES, N_PAIRS, 128, S)
    yp = yp.reshape(N_CORES, N_PAIRS, 128, T)

    key = (s_const, bias_a, c_b, d_b)
    if key not in _NC_CACHE:
        _NC_CACHE[key] = _build_core_program(*key)
    nc = _NC_CACHE[key]

    in_maps = [{"xt": xt[c], "yp": yp[c]} for c in range(N_CORES)]
    LAST_PREP = (key, in_maps)
    res = run_bass_kernel_spmd(nc, in_maps, core_ids=list(range(N_CORES)))
    LAST_RESULTS = res

    out = np.stack([res.results[c]["o"] for c in range(N_CORES)])
    return out.reshape(B, H, S, T)


if __name__ == "__main__":
    rng = np.random.default_rng(0)
    x = rng.integers(-128, 128, size=(B, H, S, D)).astype(np.int32)
    y = rng.integers(-128, 128, size=(B, H, D, T)).astype(np.int32)
    out = kernel(x=x, y=y, alpha=np.float32(0.000234), a_zp=np.float32(3.0),
                 b_zp=np.float32(-5.0), out_zp=np.float32(2.0),
                 o_alpha=np.float32(0.0625))
    print("kernel output", out.shape, out.dtype)



# revision 23
# speedup vs baseline: 92.3273x; 4.9740x over previous
"""Trainium2 Bass kernel for nn_BMMS8TS8NS8T: batched int8-valued GEMM with
dequant/requant, sharded head-parallel across 8 NeuronCores.

Reference semantics (jax CPU, fp32):
    a = x.float() - a_zp          # [B,H,S,D]  int8-valued
    b = y.float() - b_zp          # [B,H,D,T]
    q = a @ b                     # exact integers (|q| <= 64*131*132 < 2^24)
    v = fl(fl(q * s) + out_zp),   s = fl(alpha / o_alpha)
    out = trunc(clip(v, -128, 127)).astype(int8)   # trunc toward zero

Device strategy per core (12 heads = (B*H)/8, head parallel, no cross-core
communication):
  - host pre-dequantizes inputs to bf16 (exact: all values are integers with
    |v| <= 132 < 256, exactly representable in bf16) and pre-transposes /
    permutes x so the stationary matmul operand needs no on-device transpose
  - TensorE: K=64 matmuls, two heads packed in the 128-row PE array via row
    tiling (tile_position (0,0)/(64,0)); fp32 PSUM accumulation is exact
  - requantization is a two-pass scheme (exact trunc toward zero cannot be a
    single affine op + RNE convert: the trunc bin at 0 is double-width):
      pass1:  A_i16 = floor(v)   (clipped by i16 saturation far out)
      pass2:  out_i8 = sat_i8(RNE(A*(255/256) + 0.499))
                     = clip(A + [A<0], -128, 127) == trunc-clip(v)
    pass1 is the bottleneck (PSUM fp32 reads are 1x on every engine), so it
    is split across BOTH ScalarE and VectorE, and pass2 is split across
    VectorE (16-bit 2x mode) and GpSimd:
      ScalarE pass1 (fused single-rounding fma, validated exhaustively over
        every reachable q):  A = sat_i16(RNE(q*s + (zp - 0.5 + 2^-18)))
      VectorE pass1 (two fp32 roundings; (add, mult) op order so y is a
        full-mantissa product -- tie-free; validated exhaustively):
        A = sat_i16(RNE(fl(fl(q + b0) * s2))),
        b0 = fl((zp - 0.5)/s + 2^-8),  s2 = fl(s * (1 + 2^-22))
  - x^T columns are host-permuted so psum partition p owns output rows
    s = 8p+j: each partition's 8 rows form one contiguous 8 KiB DRAM run,
    keeping the output store near line rate
  - int8 results are staged in pool-aligned [128, 2048] SBUF tiles (offset
    slices of a bigger tile demote the VectorE op from 2x to 1x - measured)
"""

from contextlib import ExitStack
import numpy as np
import ml_dtypes

import concourse.bacc as bacc
import concourse.tile as tile
from concourse import mybir
from concourse import dve_ops as _dve_ops
from concourse.bass_utils import run_bass_kernel_spmd
from concourse.dve_spec import C0, C1, C2, Spec, Src0, lower
from concourse.dve_uop import DveOpSpec

AF = mybir.ActivationFunctionType
OP = mybir.AluOpType
BF16 = mybir.dt.bfloat16


def _trunc_requant_ref(in0, in1, s0, s1, imm2):
    """Faithful fp32 emulation of TRUNC_REQUANT_ANT for CoreSim."""
    f32 = np.float32
    u = (in0.astype(f32) * f32(s0)).astype(f32) + f32(s1)
    u = u.astype(f32)
    ind = (u < f32(-imm2)).astype(f32) - (u > f32(imm2)).astype(f32)
    return (u + (f32(imm2) * ind).astype(f32)).astype(f32)


def _register_trunc_op():
    """Custom DVE uop: single-pass exact requant+trunc, psum fp32 -> int8.

        u   = fl(fl(q * s2) + zp)               (s2 = s * (1 + 2^-22))
        out = sat_i8(RNE(u + 0.5*((u < -0.5) - (u > 0.5))))

    The three-zone correction shifts u by +-0.5 so RNE lands on
    trunc-toward-zero; the middle zone gives the double-width output bin at
    0 that no single affine+RNE can produce.  Validated exhaustively on the
    host over every reachable q in [-1106688, 1047552]: 0 mismatches vs the
    reference fp32 chain (including saturation; RNE half-to-even ties fixed
    by the 2^-22 scale inflation, symmetric for trunc).
    """
    name = "TRUNC_REQUANT_ANT"
    for op in _dve_ops.OPS:
        if op.name == name:
            return op
    _u = Src0 * C0 + C1
    spec = Spec(
        body=_u + C2 * ((_u < -C2) - (_u > C2)),
        reference=_trunc_requant_ref,
    )
    row = max(_dve_ops._SUB_OPCODE_FOR_NAME.values()) + 1
    assert row < 0x20
    shas = {}
    for ver in ("v3", "v4"):
        try:
            uops = lower(spec, ver=ver)
            shas[ver] = DveOpSpec(name=name, opcode=row, uops=uops,
                                  rd1_en=False).sha(ver)
        except Exception:
            pass
    op = _dve_ops.DveOp(name=name, spec=spec, subdim=False, uops_sha=shas)
    _dve_ops.OPS.append(op)
    _dve_ops.CUSTOM_DVE_SPECS[name] = spec
    _dve_ops._SUB_OPCODE_FOR_NAME[name] = row
    return op


TRUNC_OP = _register_trunc_op()

N_CORES = 8
B, H, S, D = 8, 12, 1024, 64
HEADS_PER_CORE = B * H // N_CORES          # 12
N_PAIRS = HEADS_PER_CORE // 2              # 6
M_BLOCKS = S // 128                        # 8
T = 1024

# set by kernel() for test.py / bench.py to inspect
LAST_RESULTS = None
LAST_PREP = None

_NC_CACHE = {}

# --- engine-assignment knobs (v2 structure)
# v2: 4-quad psum rotation ([128,1024] tiles, bufs=4) decouples matmul fills
# from drains (the old 2x[128,2048] ping-pong was period-bound: a tile's
# refill waited on its own ~2.1us drain, capping throughput at
# (fill+drain)/2 per group ~= 1.5us -> ~71us/iter).  Engine balance:
#   pass1 (psum fp32 -> int16, 1x on every engine; Pool has NO psum port):
#     ACT 1.2GHz (+ ~0.39us/op overhead, non-pipelined), DVE 0.96GHz.
#   pass2 (int16 -> int8): DVE scalar_tensor_tensor (A<0)+A at 16-bit
#     2x_1P (never grabs the DVE/Pool shared SBUF port pair), Pool
#     tensor_scalar (c_b,d_b) ~1x.
ACT_G_OF_48 = 25        # of the 48 groups, how many take the ScalarE
                        # two-pass path (ACT pass1 int16 + Pool pass2);
                        # the rest use the fused single-pass custom DVE op
                        # (TRUNC_REQUANT_ANT, psum fp32 -> int8 directly)
DEFER_N = 2             # how many groups the pass2 trails its pass1
NH_SPLIT = 2            # rhs blocks per ji (N=512 moving operand)
SPLIT_FIRST_LOAD = True

# bench-only ablation knobs (correctness-invalid when set; used by ab.py to
# attribute loop-slope time to pipeline stages)
BENCH_NO_STORE = False
BENCH_NO_LOAD = False
BENCH_NO_MM = False
BENCH_NO_P2 = False
BENCH_NO_P1 = False


def _build_core_program(s_const: float, bias_a: float, c_b: float, d_b: float,
                        loop_iters: int | None = None,
                        bench_io: bool = False):
    """One NeuronCore's program: 12 heads of [1024,64]@[64,1024] + requant.

    loop_iters: when set, wraps the whole body in a hardware For_i loop -
    used only for benchmarking (device time scales with the loop count so a
    slope isolates HW exec time from host/relay dispatch overhead).

    bench_io: all big DRAM tensors become Internal (device-resident garbage,
    never shipped over the axon relay) and a tiny [1,16] ExternalOutput is
    added so PJRT has something to return.  Timing is data-independent, so
    the loop slope is unaffected; per-call payload drops from ~15 MB/core to
    16 bytes.
    """
    nc = bacc.Bacc("TRN2", target_bir_lowering=False, debug=False)
    io_kind_in = "Internal" if bench_io else "ExternalInput"
    io_kind_out = "Internal" if bench_io else "ExternalOutput"
    # head-pairs stacked on the partition axis
    d_xt = nc.dram_tensor("xt", [N_PAIRS, 128, S], BF16, kind=io_kind_in)
    d_yp = nc.dram_tensor("yp", [N_PAIRS, 128, T], BF16, kind=io_kind_in)
    d_o = nc.dram_tensor("o", [HEADS_PER_CORE, S, T], mybir.dt.int8,
                         kind=io_kind_out)
    d_tick = (nc.dram_tensor("tick", [1, 16], mybir.dt.int8,
                             kind="ExternalOutput") if bench_io else None)

    with tile.TileContext(nc) as tc:
        with ExitStack() as stk:
            if loop_iters is not None:
                # PE's body exceeds one IRAM block; hint the back-edge so the
                # benchmark loop doesn't pay a ~3-4 us ifetch per iteration
                # that single-shot execution would not pay.
                stk.enter_context(tc.For_i(0, loop_iters, 1,
                                           hint_engines=(mybir.EngineType.PE,)))
            _emit_body(nc, tc, d_xt, d_yp, d_o, s_const, bias_a, c_b, d_b)
        if d_tick is not None:
            with tc.tile_pool(name="tick", bufs=1) as tkpool:
                tk = tkpool.tile([1, 16], mybir.dt.int8, tag="tick")
                nc.vector.memset(tk[:], 0)
                nc.sync.dma_start(d_tick[:, :], tk[:])
    nc.compile()
    return nc


def _emit_body(nc, tc, d_xt, d_yp, d_o, s_const, bias_a, c_b, d_b):
    # VectorE one-op pass1 constants (validated in validate_requant.py over
    # every reachable q: 0 mismatches, max tie margin variant)
    s64 = np.float64(np.float32(s_const))
    zp64 = np.float64(np.float32(bias_a)) + 0.5 - 2.0 ** -18  # recover out_zp
    b0 = float(np.float32((zp64 - 0.5) / s64 + 2.0 ** -8))
    s2 = float(np.float32(s64 * (1.0 + 2.0 ** -22)))

    # fused-op constants (exhaustively validated, see _register_trunc_op)
    s2c = float(np.float32(np.float64(np.float32(s_const)) *
                           (1.0 + 2.0 ** -22)))
    b2c = float(np.float32(np.float64(np.float32(bias_a)) + 0.5 - 2.0 ** -18))

    gidx = 0            # group counter (0..47)
    act_g = 0
    pending = []
    with tc.tile_pool(name="xin", bufs=2) as xpool, \
         tc.tile_pool(name="yin", bufs=2) as ypool, \
         tc.tile_pool(name="aint", bufs=6) as apool, \
         tc.tile_pool(name="obuf", bufs=4) as opool, \
         tc.tile_pool(name="ps", bufs=4, space="PSUM") as pspool:
        tiles = [None] * N_PAIRS
        xt0 = xpool.tile([128, S], BF16, tag="xt")
        yp0 = ypool.tile([128, T], BF16, tag="yp")
        if BENCH_NO_LOAD:
            # bench-only: no HBM input traffic; one memset pair feeds every
            # head-pair (DVE ~1.6us/iter pollution, noted in readings)
            nc.vector.memset(xt0[:], 0)
            nc.vector.memset(yp0[:], 0)
        elif SPLIT_FIRST_LOAD:
            # single-shot prologue: land group-0's operands first so the
            # first matmuls (and ScalarE) start ~2 us earlier
            nc.sync.dma_start(xt0[:, 0:256], d_xt[0, :, 0:256])
            nc.sync.dma_start(yp0[:], d_yp[0, :, :])
            nc.sync.dma_start(xt0[:, 256:S], d_xt[0, :, 256:S])
        else:
            nc.sync.dma_start(xt0[:], d_xt[0, :, :])
            nc.sync.dma_start(yp0[:], d_yp[0, :, :])
        tiles[0] = (xt0, yp0)

        ob_dummy = None
        if BENCH_NO_P2 or BENCH_NO_P1:
            # bench-only: stores read this pre-written tile (keeps the HBM
            # store traffic while removing the producing ops)
            ob_dummy = opool.tile([128, 2048], mybir.dt.int8, tag="obs0")
            nc.vector.memset(ob_dummy[:], 0)

        ps_dummy = None
        if BENCH_NO_MM:
            # bench-only: pass1 reads this one pre-written quad (keeps the
            # psum-read cost while removing the matmul fills)
            ps_dummy = pspool.tile([128, 1024], mybir.dt.float32, tag="ps")
            nc.vector.memset(ps_dummy[:], 0)

        for pair in range(N_PAIRS):
            xt_t, yp_t = tiles[pair]
            if pair + 1 < N_PAIRS:
                # prefetch next pair's operands now so the loads sit ahead
                # of this pair's output stores in the SP HWDGE FIFO
                if BENCH_NO_LOAD:
                    tiles[pair + 1] = (xt0, yp0)
                else:
                    xt_n = xpool.tile([128, S], BF16, tag="xt")
                    yp_n = ypool.tile([128, T], BF16, tag="yp")
                    nc.sync.dma_start(xt_n[:], d_xt[pair + 1, :, :])
                    nc.sync.dma_start(yp_n[:], d_yp[pair + 1, :, :])
                    tiles[pair + 1] = (xt_n, yp_n)

            ob = {}

            for jg in range(M_BLOCKS // 2):
                for h2 in range(2):
                    # per-group engine split: ACT two-pass (+Pool pass2) vs
                    # fused single-pass custom DVE op straight to int8
                    want_act = ((gidx + 1) * ACT_G_OF_48) // 48 > act_g
                    gidx += 1
                    if want_act:
                        act_g += 1
                    a_t = (apool.tile([128, 2048], mybir.dt.int16,
                                      tag="a", name=f"a_{pair}_{h2}_{jg}")
                           if want_act and not BENCH_NO_P1 else None)
                    no_writer = (BENCH_NO_P1 or (want_act and BENCH_NO_P2))
                    ob_t = (ob_dummy if no_writer else
                            opool.tile([128, 2048], mybir.dt.int8,
                                       tag=f"obs{jg}",
                                       name=f"ob_{pair}_{h2}_{jg}"))
                    ob[(h2, jg)] = ob_t
                    for ji in range(2):
                        j = jg * 2 + ji
                        lhsT = xt_t[64 * h2:64 * h2 + 64,
                                    j * 128:(j + 1) * 128]
                        psj = (ps_dummy if BENCH_NO_MM else
                               pspool.tile([128, 1024], mybir.dt.float32,
                                           tag="ps",
                                           name=f"ps_{pair}_{h2}_{jg}_{ji}"))
                        for nh in (() if BENCH_NO_MM else range(NH_SPLIT)):
                            nw = 1024 // NH_SPLIT
                            nc.tensor.matmul(
                                psj[:, nh * nw:(nh + 1) * nw],
                                lhsT,
                                yp_t[64 * h2:64 * h2 + 64,
                                     nh * nw:(nh + 1) * nw],
                                start=True, stop=True,
                                tile_position=(64 * h2, 0),
                            )
                        # drain this quad now; 4-buffer rotation keeps the
                        # fills decoupled
                        if BENCH_NO_P1:
                            pass
                        elif want_act:
                            nc.scalar.activation(
                                a_t[:, ji * 1024:(ji + 1) * 1024], psj[:],
                                AF.Copy, bias=bias_a, scale=s_const)
                        else:
                            nc.vector._custom_dve(
                                TRUNC_OP,
                                out=ob_t[:, ji * 1024:(ji + 1) * 1024],
                                in0=psj[:], s0=s2c, s1=b2c, imm2=0.5)
                    # Pool pass2 for ACT-groups only, deferred so it trails
                    # pass1 by DEFER_N groups
                    if want_act and not BENCH_NO_P1 and not BENCH_NO_P2:
                        def fmap(a_t=a_t, ob_t=ob_t):
                            nc.gpsimd.tensor_scalar(ob_t[:], a_t[:],
                                                    c_b, d_b,
                                                    OP.mult, OP.add)
                        pending.append(fmap)
                    while len(pending) > DEFER_N:
                        pending.pop(0)()
            # flush this pair's remaining pass2 ops, then batched output DMAs
            while pending:
                pending.pop(0)()
            for h2 in range(2):
                dst = d_o[2 * pair + h2, :, :].rearrange(
                    "(p j) t -> p (j t)", j=M_BLOCKS)
                for jg in range(M_BLOCKS // 2):
                    if not BENCH_NO_STORE:
                        nc.sync.dma_start(dst[:, jg * 2048:(jg + 1) * 2048],
                                          ob[(h2, jg)][:])


def default_key():
    """Requant constants for the reference problem's quantization params -
    used by bench.py when kernel() hasn't run in this process."""
    s_const = float(np.float32(np.float32(0.000234) / np.float32(0.0625)))
    bias_a = float(np.float64(np.float32(2.0)) - 0.5 + 2.0 ** -18)
    c_b = float(np.float32(255.0 / 256.0))
    d_b = float(np.float32(0.499))
    return (s_const, bias_a, c_b, d_b)


def kernel(x, y, alpha, a_zp, b_zp, out_zp, o_alpha):
    global LAST_RESULTS, LAST_PREP
    x = np.asarray(x)
    y = np.asarray(y)
    s_const = float(np.float32(np.float32(alpha) / np.float32(o_alpha)))
    bias_a = float(np.float64(np.float32(out_zp)) - 0.5 + 2.0 ** -18)
    c_b = float(np.float32(255.0 / 256.0))
    d_b = float(np.float32(0.499))

    # ---- host-side shard + dequant prep (exact in bf16) ----
    xf = x.reshape(B * H, S, D).astype(np.float32) - np.float32(a_zp)
    yf = y.reshape(B * H, D, T).astype(np.float32) - np.float32(b_zp)
    # lhsT layout: [head, D, S], head-pairs stacked to 128 partitions.
    # S-columns permuted to c = j*128 + p  <->  s = 8p + j so each psum
    # partition owns 8 consecutive output rows (8 KiB DMA runs).
    xt = np.ascontiguousarray(xf.transpose(0, 2, 1)).astype(ml_dtypes.bfloat16)
    xt = np.ascontiguousarray(
        xt.reshape(B * H, D, S // 8, 8).transpose(0, 1, 3, 2)).reshape(
        B * H, D, S)
    yp = yf.astype(ml_dtypes.bfloat16)
    xt = xt.reshape(N_CORES, N_PAIRS, 128, S)
    yp = yp.reshape(N_CORES, N_PAIRS, 128, T)

    key = (s_const, bias_a, c_b, d_b)
    if key not in _NC_CACHE:
        _NC_CACHE[key] = _build_core_program(*key)
    nc = _NC_CACHE[key]

    in_maps = [{"xt": xt[c], "yp": yp[c]} for c in range(N_CORES)]
    LAST_PREP = (key, in_maps)
    res = run_bass_kernel_spmd(nc, in_maps, core_ids=list(range(N_CORES)))
    LAST_RESULTS = res

    out = np.stack([res.results[c]["o"] for c in range(N_CORES)])
    return out.reshape(B, H, S, T)


if __name__ == "__main__":
    rng = np.random.default_rng(0)
    x = rng.integers(-128, 128, size=(B, H, S, D)).astype(np.int32)
    y = rng.integers(-128, 128, size=(B, H, D, T)).astype(np.int32)
    out = kernel(x=x, y=y, alpha=np.float32(0.000234), a_zp=np.float32(3.0),
                 b_zp=np.float32(-5.0), out_zp=np.float32(2.0),
                 o_alpha=np.float32(0.0625))
    print("kernel output", out.shape, out.dtype)

